# revision 74
# baseline (speedup 1.0000x reference)
"""GroupedQueryAttention Trainium2 kernel (bf16 + one-sided-fp8 S).

Sharding: 8 cores = 2 (batch) x 4 (kv-head groups / tensor parallel).
Core c: b = c//4, g = c%4 owns q-heads 4g..4g+3 and kv-head g.
Each core computes a partial o-projection (its 512 rows of Wo); the host
sums the 4 partials per batch (the "all-reduce" of the TP group).

Design notes (cost-model driven; ~188us of PE busy, the roofline for
this decomposition):
  - Everything bf16 on the wire and in SBUF (PSUM fp32): halves DMA
    traffic/SBUF pressure, and matmuls run 1 cyc/row at any ap size.
  - S matmuls in fp8 e4m3 DoubleRow (0.5 cyc/row): k is stored as an
    fp8 hi+lo pair occupying the two DoubleRow weight slots (so k
    carries NO quantization error), q as a single fp8 tile broadcast
    into both rhs slots via a 0-stride AP. q/k are pre-scaled by 16 on
    the host (fp8 range centering); the exp undoes the 256x with its
    scale operand. Measured end-to-end rel-err ~1.45e-2 (gate 2e-2);
    fp8 anywhere else (AV/proj/o-proj, even one-sided) breaks the gate.
  - v projected directly into natural [s, d] layout (lhsT = x block,
    rhs = Wv tile, ap=128): no PE transposes. All 4 column chains share
    ONE psum accumulation group (a bank holds one pending-zero region).
  - causal mask accumulated ON THE PE: trineg^T @ tri2 = -1e30*(s-t)
    for t < s, added into the diagonal S block - exp follows the S
    matmul immediately with no cross-engine mask hop.
  - RoPE rotate-half via a PE permutation matmul (swapid). DVE cannot
    cross partitions on real HW (CoreSim permits it; the BIR verifier
    does not). The rot matmul is emitted 2 feeder slots after its evac
    copy so the PE never waits on the copy engine.
  - each head's final den/AV block + reciprocal + O^T evac are deferred
    into the NEXT head's stream; av is evacuated unnormalized (frees
    the PSUM bank without waiting the ~5us 1/den DRAM-round-trip
    broadcast) and normalized in place later.
  - o-proj tail fully pipelined: (h0..h2) partials of the last 4
    t-tiles stage to SBUF mid-chunk-3; h3/chunk-3's AV + denominator
    close per 128-column block at s-blocks 12..15, so each tail row
    runs inside h3's attention as one matmul + one fused DVE
    scalar_tensor_tensor (yr = yp*(1/den) + y01).
  - DMA: ~4KB/partition pieces spread over SP/Act HWDGE and the Pool
    SWDGE path (which bypasses both the SP sequencer serialization and
    the shared HWDGE generator); y staged per t-row; final row flushed
    per-nch on two queues so the last transfer is small.
"""

import math
import sys

import ml_dtypes
import numpy as np

sys.path.insert(0, "/opt/trn_rl_repo")

import concourse.bass as bass  # noqa: E402
import concourse.tile as tile  # noqa: E402
from concourse import bacc, mybir  # noqa: E402
from concourse.bass_utils import run_bass_kernel_spmd  # noqa: E402

B, T, D = 2, 2048, 2048
NH, NKV, HD = 16, 4, 128
NQ = NH // NKV  # q heads per core
KC = D // 128  # contraction chunks
NT = T // 128  # t tiles
NJ = T // 512  # t chunks
F32 = mybir.dt.float32
BF16 = mybir.dt.bfloat16
F8 = mybir.dt.float8e4
DR = mybir.MatmulPerfMode.DoubleRow
X = mybir.AxisListType.X
EXP = mybir.ActivationFunctionType.Exp
COPY = mybir.ActivationFunctionType.Copy
BFNP = ml_dtypes.bfloat16
# q/k are pre-scaled by 16 on the host (fp8 range centering); S comes out
# scaled by 256, undone by the exp's scale operand.
SEXP = 1.0 / 256.0


def _body(tc, xt, wq, wk, wv, wo, cost_d, sint_d, maskm_d, y_d):
    nc = tc.nc
    from contextlib import ExitStack

    with ExitStack() as ctx:
        consts = ctx.enter_context(tc.tile_pool(name="consts", bufs=1))
        wpool = ctx.enter_context(tc.tile_pool(name="wpool", bufs=10))
        seq = ctx.enter_context(tc.tile_pool(name="seq", bufs=1))
        blk = ctx.enter_context(tc.tile_pool(name="blk", bufs=2))
        ptp = ctx.enter_context(tc.tile_pool(name="ptp", bufs=5))
        rt = ctx.enter_context(tc.tile_pool(name="rt", bufs=2))
        invp = ctx.enter_context(tc.tile_pool(name="invp", bufs=4))
        dram = ctx.enter_context(tc.tile_pool(name="dram", bufs=4, space="DRAM"))
        ps = ctx.enter_context(tc.tile_pool(name="ps", bufs=3, space="PSUM"))

        # causal mask as a PE accumulation: trineg^T @ tri2 adds
        # -1e30*(s-t) to the diagonal block where t < s, 0 elsewhere
        trineg = consts.tile([128, 128], BF16, tag="trineg")
        tri2 = consts.tile([128, 128], BF16, tag="tri2")
        swapid = consts.tile([128, 128], BF16, tag="swapid")
        onesr = consts.tile([128, 2], BF16, tag="onesr")
        nc.vector.memset(onesr, 1.0)

        wkt = wpool.tile([128, 16, 128], BF16, tag="w", name="wkt")
        wvt = wpool.tile([128, 16, 128], BF16, tag="w", name="wvt")
        wqt = [
            wpool.tile([128, 4, 512], BF16, tag="w", name=f"wq{i}") for i in range(4)
        ]
        wkr = wk.rearrange("p (c m) -> p c m", m=128)
        wvr = wv.rearrange("p (c m) -> p c m", m=128)
        cost = consts.tile([128, T], BF16, tag="cost")
        sint = consts.tile([128, T], BF16, tag="sint")

        # chunk-0 x: one [128, 16, 512] tile, loaded in staggered pieces so
        # the prologue chains (k lag 0, v lag 2, q lag 6) track the stream.
        # Small first pieces + SP/Act/Pool queue spreading minimize the time
        # to the first matmul (~3.9us: seq + HWDGE + DGE + transfer + sem).
        xw0 = blk.tile([128, 16, 512], BF16, tag="blk", name="xw0")
        xr0 = xt.rearrange("(c p) m -> p c m", p=128)[:, :, 0:512]
        wqr = [
            wq[:, 2048 * g : 2048 * (g + 1)].rearrange("p (c m) -> p c m", m=512)
            for g in range(4)
        ]
        # Pool's SWDGE path bypasses the shared HWDGE descriptor generator,
        # which serializes at ~625ns/DMA and is the real start constraint.
        # x pieces ride the Pool SWDGE path: it bypasses both the SP
        # sequencer serialization (~1.2us/DMA) and the shared HWDGE
        # descriptor generator (~625ns/DMA)
        nc.sync.dma_start(wkt[:, 0:2, :], wkr[:, 0:2, :])
        nc.scalar.dma_start(xw0[:, 0:2, :], xr0[:, 0:2, :])
        nc.gpsimd.dma_start(xw0[:, 2:4, :], xr0[:, 2:4, :])
        nc.sync.dma_start(wkt[:, 2:16, :], wkr[:, 2:16, :])
        nc.gpsimd.dma_start(wvt[:, 0:4, :], wvr[:, 0:4, :])
        nc.sync.dma_start(xw0[:, 4:8, :], xr0[:, 4:8, :])
        nc.sync.dma_start(wqt[0], wqr[0])
        nc.gpsimd.dma_start(xw0[:, 8:12, :], xr0[:, 8:12, :])
        nc.sync.dma_start(wvt[:, 4:16, :], wvr[:, 4:16, :])
        nc.gpsimd.dma_start(xw0[:, 12:16, :], xr0[:, 12:16, :])
        nc.scalar.dma_start(wqt[1], wqr[1])
        nc.gpsimd.dma_start(wqt[2], wqr[2])
        nc.gpsimd.dma_start(wqt[3], wqr[3])
        nc.scalar.dma_start(trineg, maskm_d[0:128, :])
        nc.scalar.dma_start(tri2, maskm_d[128:256, :])
        nc.scalar.dma_start(swapid, maskm_d[256:384, :])
        nc.sync.dma_start(cost, cost_d)
        nc.sync.dma_start(sint, sint_d)

        # Activation-table preload: a dummy exp right after the DMA configs
        # so the one-time 1.28us table load runs during the initial stream-in
        # instead of in front of the first real exp.
        scratch = consts.tile([128, 1], F32, tag="scr")
        nc.vector.memset(scratch, 0.0)
        nc.scalar.activation(scratch, scratch, EXP)

        # q in single fp8 (DoubleRow rhs broadcasts it to both slots);
        # k as an fp8 hi+lo pair occupying the two DoubleRow weight slots,
        # so S matmuls run at 0.5 cycles/row with only q's quantization error
        qT8 = [seq.tile([128, T], F8, tag=f"qT{h}", name=f"qT{h}") for h in range(NQ)]
        kT8 = seq.tile([128, 2, T], F8, tag="kT", name="kT8")
        OT = [seq.tile([128, T], BF16, tag=f"ot{h}", name=f"ot{h}") for h in range(NQ)]
        vnat = seq.tile([128, T], BF16, tag="vnat", name="vnat")
        wot = []

        def wslc(m, kc):
            if m == "k":
                return wkt[:, kc, :]
            h = int(m[1])
            return wqt[kc // 4][:, kc % 4, 128 * h : 128 * (h + 1)]

        # ---------- emission units (proj chains, evacs, o-proj tiles) ------
        def chain_part(j, m, ref, xw, lo, hi):
            def emit():
                if lo == 0:
                    ref["pm"] = ps.tile(
                        [128, 512], F32, tag="pm", bufs=2, name=f"pm{j}_{m}"
                    )
                pm = ref["pm"]
                for kc in range(lo, hi):
                    if m == "v":
                        # one psum group for all 4 column chains (a bank can
                        # hold only one pending-zero region at a time)
                        for c in range(4):
                            nc.tensor.matmul(
                                pm[:, 128 * c : 128 * (c + 1)],
                                xw[:, kc, 128 * c : 128 * (c + 1)],
                                wvt[:, kc, :],
                                start=(kc == 0 and c == 0),
                                stop=(kc == KC - 1 and c == 3),
                            )
                    else:
                        nc.tensor.matmul(
                            pm,
                            wslc(m, kc),
                            xw[:, kc, :],
                            start=(kc == 0),
                            stop=(kc == KC - 1),
                        )
            return emit

        def rope_copy(j, m, ref, act=True):
            # always on Act: the rot matmul (PE) waits on this copy, and the
            # DVE queue is the congested one mid-chunk
            def emit():
                pm = ref["pm"]
                t0 = rt.tile([128, 512], BF16, tag="t0", bufs=6, name=f"t0_{j}_{m}")
                nc.scalar.activation(t0, pm, COPY)
                ref["t0"] = t0
            return emit

        def rope_rest(j, m, ref):
            # separate feeder unit: the rot matmul lands one injection slot
            # after the copy, so the PE never waits on the copy's engine
            def emit():
                ch = slice(512 * j, 512 * (j + 1))
                t0 = ref["t0"]
                # rotate-half on the PE (the DVE cannot cross partitions);
                # the sign of the rotation is folded into sint
                rot = ps.tile([128, 512], F32, tag="pm", bufs=2, name=f"ro{j}_{m}")
                nc.tensor.matmul(rot, swapid, t0)
                tc_ = rt.tile([128, 512], BF16, tag="t2", bufs=6, name=f"tc_{j}_{m}")
                nc.gpsimd.tensor_mul(tc_, t0, cost[:, ch])
                tmp = rt.tile([128, 512], BF16, tag="t1", bufs=6, name=f"t1_{j}_{m}")
                nc.vector.tensor_mul(tmp, rot, sint[:, ch])
                if m == "k":
                    ktb = rt.tile([128, 512], BF16, tag="ktb", bufs=2, name=f"kb{j}")
                    nc.vector.tensor_add(ktb, tc_, tmp)
                    # fp8 hi/lo split into the two DoubleRow weight slots
                    # (all DVE: keeps the Act queue clear for the exp stream)
                    nc.vector.tensor_copy(kT8[:, 0, ch], ktb)
                    ktr = rt.tile([128, 512], BF16, tag="ktr", bufs=2, name=f"kr{j}")
                    nc.vector.tensor_sub(ktr, ktb, kT8[:, 0, ch])
                    nc.vector.tensor_copy(kT8[:, 1, ch], ktr)
                else:
                    nc.vector.tensor_add(qT8[int(m[1])][:, ch], tc_, tmp)
            return emit

        def rope_evac(j, m, ref, act=False):
            def emit():
                rope_copy(j, m, ref, act)()
                rope_rest(j, m, ref)()
            return emit

        def v_evac(j, ref):
            def emit():
                ch = slice(512 * j, 512 * (j + 1))
                nc.vector.tensor_copy(vnat[:, ch], ref["pm"])
            return emit

        def wo_load(hh):
            def emit():
                w = wpool.tile([128, T], BF16, tag="w", name=f"wo{hh}")
                nc.sync.dma_start(w, wo[128 * hh : 128 * (hh + 1), :])
                wot.append(w)
            return emit

        # y staged per t-row; one 4KB/partition DMA per row on the Act queue
        yrow = {}

        def get_yrow(it):
            if it not in yrow:
                yrow[it] = rt.tile([128, T], BF16, tag="yrow", bufs=5, name=f"yr{it}")
            return yrow[it]

        def yflush(it):
            # SP is idle during chunk 3 (no x loads): row flushes go there
            nc.sync.dma_start(y_d[128 * it : 128 * (it + 1), :], yrow.pop(it))

        def oproj_tile(it, nch):
            def emit():
                yp = ps.tile([128, 512], F32, tag="pm", bufs=2, name=f"yp{it}_{nch}")
                for hh in range(4):
                    nc.tensor.matmul(
                        yp,
                        OT[hh][:, 128 * it : 128 * (it + 1)],
                        wot[hh][:, 512 * nch : 512 * (nch + 1)],
                        start=(hh == 0),
                        stop=(hh == 3),
                    )
                yr = get_yrow(it)
                # 3:1 Act/DVE split: late chunk-3 DVE is congested with the
                # pipelined tail's fused adds while Act's exps are tapering
                if nch != 3:
                    nc.scalar.activation(yr[:, 512 * nch : 512 * (nch + 1)], yp, COPY)
                else:
                    nc.vector.tensor_copy(yr[:, 512 * nch : 512 * (nch + 1)], yp)
                if nch == 3:
                    yflush(it)
            return emit

        # tail split: (h0,h1,h2) partials of the last 4 t-tiles staged to
        # SBUF mid-chunk-3; the h3 term is pipelined INTO h3's attention:
        # h3/chunk-3's AV and denominator complete per 128-column block at
        # s-blocks 12..15, so each tail row runs right after its block
        # closes, fused as yr = yp*(1/den) + y01 in one DVE op.
        y01 = {}

        def oproj01_tile(it, nch):
            def emit():
                yp = ps.tile([128, 512], F32, tag="pm", bufs=2, name=f"ya{it}_{nch}")
                for hh in range(3):
                    nc.tensor.matmul(
                        yp,
                        OT[hh][:, 128 * it : 128 * (it + 1)],
                        wot[hh][:, 512 * nch : 512 * (nch + 1)],
                        start=(hh == 0),
                        stop=(hh == 2),
                    )
                if nch % 2 == 0:
                    nc.scalar.activation(y01[(it, nch)], yp, COPY)
                else:
                    nc.vector.tensor_copy(y01[(it, nch)], yp)
            return emit
            return emit

        def proj_units(j, xw):
            # each chain's rope tail (rot matmul etc.) is deferred past the
            # NEXT chain's first part so its evac copy has ~2 injection
            # slots of lead before the PE needs it
            units = []
            pending = None
            for m in ["k", "v", "q0", "q1", "q2", "q3"]:
                ref = {}
                for i, lo in enumerate(range(0, KC, 4)):
                    units.append(chain_part(j, m, ref, xw, lo, lo + 4))
                    if i == 0 and pending is not None:
                        units.append(pending)
                        pending = None
                if m == "v":
                    units.append(v_evac(j, ref))
                else:
                    units.append(rope_copy(j, m, ref))
                    pending = rope_rest(j, m, ref)
                if j == 2:
                    if m == "k":
                        units.append(wo_load(0))
                    elif m == "v":
                        units.append(wo_load(1))
                    elif m == "q0":
                        units.append(wo_load(2))
                    elif m == "q1":
                        units.append(wo_load(3))
            units.append(pending)
            return units

        # ---------- prologue: proj(0) chains interleaved with the x/w
        # stream-in; q1/q2/q3 borrow the idle av/den PSUM tags
        MS = ["k", "v", "q0", "q1", "q2", "q3"]
        QS = ["q0", "q1", "q2", "q3"]
        ptag = {"k": ("pm", 2), "v": ("pm", 2), "q0": ("ps", 2),
                "q1": ("av", 2), "q2": ("av", 2), "q3": ("den", 2)}
        refs = {m: {} for m in MS}
        for m in MS:
            tg, nb = ptag[m]
            refs[m]["pm"] = ps.tile(
                [128, 512], F32, tag=tg, bufs=nb, name=f"pm0_{m}"
            )

        def mm0(m, kc):
            pm = refs[m]["pm"]
            if m == "v":
                for c in range(4):
                    nc.tensor.matmul(
                        pm[:, 128 * c : 128 * (c + 1)],
                        xw0[:, kc, 128 * c : 128 * (c + 1)],
                        wvt[:, kc, :],
                        start=(kc == 0 and c == 0),
                        stop=(kc == KC - 1 and c == 3),
                    )
            else:
                nc.tensor.matmul(
                    pm,
                    wslc(m, kc),
                    xw0[:, kc, :],
                    start=(kc == 0),
                    stop=(kc == KC - 1),
                )

        # k lags 0, v lags 5, q lags 9 behind the x/w arrival stream
        for kc in range(KC):
            mm0("k", kc)
            if kc >= 5:
                mm0("v", kc - 5)
            if kc >= 9:
                for m in QS:
                    mm0(m, kc - 9)
        # chain-major flush: each chain's rope evac is emitted the moment
        # its chain closes, overlapping the remaining flush matmuls, so
        # kT8/qT8 are ready well before attention starts. (k's rot must
        # follow v's evac: the pm ring would otherwise make the rot wait
        # on v's still-accumulating psum from inside the PE queue.)
        for kc in range(KC - 5, KC):
            mm0("v", kc)
        v_evac(0, refs["v"])()
        rope_evac(0, "k", refs["k"], act=True)()
        prev_rest = None
        for m in QS:
            for kc in range(KC - 9, KC):
                mm0(m, kc)
            if prev_rest is not None:
                prev_rest()
            rope_copy(0, m, refs[m])()
            prev_rest = rope_rest(0, m, refs[m])
        prev_rest()
        pro_evacs = []

        # ---------- attention per chunk, feeder interleaves next-chunk work
        pend = []
        for j in range(NJ):
            ch = slice(512 * j, 512 * (j + 1))
            nst = 4 * j + 4

            if j < 3:
                xw = blk.tile([128, 16, 512], BF16, tag="blk", name=f"xw{j + 1}")
                xr = xt.rearrange("(c p) m -> p c m", p=128)[
                    :, :, 512 * (j + 1) : 512 * (j + 2)
                ]
                for piece in range(4):
                    nc.sync.dma_start(
                        xw[:, 4 * piece : 4 * piece + 4, :],
                        xr[:, 4 * piece : 4 * piece + 4, :],
                    )
                feed = proj_units(j + 1, xw)
                if j == 0:
                    feed = pro_evacs + feed
            else:
                # (h0..h2) partials of the tail t-tiles sit last in the
                # feed: OT2's chunk-3 columns are ready early in h3, where
                # these inject (h3 runs inject(2) per s-block). y01 tiles
                # are pre-allocated so the pipelined tail can reference
                # them before the writes are emitted.
                for it in range(12, NT):
                    for nch in range(4):
                        y01[(it, nch)] = rt.tile(
                            [128, 512], BF16, tag="y01", bufs=16,
                            name=f"yA{it}_{nch}",
                        )
                op = [oproj_tile(it, nch) for it in range(12) for nch in range(4)]
                ya = [oproj01_tile(it, nch) for it in range(12, NT) for nch in range(4)]
                feed = op + ya

            def inject(n):
                for _ in range(n):
                    if feed:
                        feed.pop(0)()

            for h in range(NQ):
                den8 = ps.tile(
                    [128, 8],
                    F32,
                    tag="den",
                    bufs=2,
                    padded_shape=[128, 512],
                    name=f"den{h}_{j}",
                )
                av = ps.tile([128, 512], F32, tag="av", bufs=2, name=f"av{h}_{j}")
                pts = [None] * nst

                def s_block(st):
                    off = 128 * (st - 4 * j)
                    lo = max(0, off)
                    w = 512 - lo
                    sps = ps.tile(
                        [128, 512], F32, tag="ps", bufs=2, name=f"s{h}_{j}_{st}"
                    )
                    qb = (
                        qT8[h][:, 512 * j + lo : 512 * (j + 1)]
                        .rearrange("p (o m) -> p o m", o=1)
                        .broadcast_to([128, 2, w])
                    )
                    nc.tensor.matmul(
                        sps[:, lo:512],
                        kT8[:, :, 128 * st : 128 * (st + 1)],
                        qb,
                        perf_mode=DR,
                        start=True,
                        stop=(off < 0),
                    )
                    if off >= 0:
                        # causal mask accumulated on the PE: adds
                        # -1e30*(s-t) where t < s inside the diagonal block
                        nc.tensor.matmul(
                            sps[:, off : off + 128],
                            trineg,
                            tri2,
                            start=False,
                            stop=True,
                        )
                    pt = ptp.tile([128, 512], BF16, tag="pt", name=f"pt{h}_{j}_{st}")
                    nc.scalar.activation(pt[:, lo:512], sps[:, lo:512], EXP, scale=SEXP)
                    pts[st] = pt

                lasth = j == 3 and h == 3

                def den_av(st, den8=den8, av=av, pts=pts, nst=nst, lasth=lasth):
                    for c in range(max(0, st - 4 * j), 4):
                        # for the pipelined last head, each den column chain
                        # closes at its own final contribution (st = 12+c)
                        stp = (
                            (st == 4 * j + c)
                            if lasth
                            else (st == nst - 1 and c == 3)
                        )
                        nc.tensor.matmul(
                            den8[:, 2 * c : 2 * c + 2],
                            pts[st][:, 128 * c : 128 * (c + 1)],
                            onesr,
                            start=(st == 0 and c == 0),
                            stop=stp,
                        )
                    c0 = max(0, 128 * (st - 4 * j))
                    nc.tensor.matmul(
                        av[:, c0:512],
                        vnat[:, 128 * st : 128 * (st + 1)],
                        pts[st][:, c0:512],
                        start=(st == 0),
                        stop=(st == nst - 1),
                    )

                def tail_block(b, den8=den8, av=av, ch=ch):
                    # one 128-token row of the j3/h3 o-proj tail: available
                    # as soon as den chain b and av columns [128b..] close
                    it = 12 + b
                    invb = rt.tile([128, 1], F32, tag="d4", bufs=3, name=f"i3_{b}")
                    nc.vector.reciprocal(invb, den8[:, 2 * b : 2 * b + 1])
                    # Act copy: the DVE queue holds the previous block's
                    # fused adds, and the yb matmuls wait on this evac
                    nc.scalar.activation(
                        OT[3][:, 128 * it : 128 * (it + 1)],
                        av[:, 128 * b : 128 * (b + 1)],
                        COPY,
                    )
                    for nch in range(4):
                        yp = ps.tile(
                            [128, 512], F32, tag="pm", bufs=2, name=f"yb{it}_{nch}"
                        )
                        nc.tensor.matmul(
                            yp,
                            OT[3][:, 128 * it : 128 * (it + 1)],
                            wot[3][:, 512 * nch : 512 * (nch + 1)],
                        )
                        yr = get_yrow(it)
                        sl = slice(512 * nch, 512 * (nch + 1))
                        nc.vector.scalar_tensor_tensor(
                            yr[:, sl],
                            yp,
                            invb,
                            y01[(it, nch)],
                            mybir.AluOpType.mult,
                            mybir.AluOpType.add,
                        )
                        if it < 15:
                            if nch == 3:
                                nc.sync.dma_start(
                                    y_d[128 * it : 128 * (it + 1), :], yrow.pop(it)
                                )
                        else:
                            # final row: per-nch flushes split over SP/Act
                            if nch % 2 == 0:
                                nc.sync.dma_start(
                                    y_d[128 * it : 128 * (it + 1), sl], yr[:, sl]
                                )
                            else:
                                nc.scalar.dma_start(
                                    y_d[128 * it : 128 * (it + 1), sl], yr[:, sl]
                                )
                            if nch == 3:
                                yrow.pop(it)

                def make_tail(h=h, j=j, nst=nst, den8=den8, av=av,
                              den_av=den_av, ch=ch, tail_block=tail_block,
                              lasth=lasth):
                    def emit():
                        den_av(nst - 1)
                        if lasth:
                            # final 128-token block of the pipelined tail
                            tail_block(3)
                            return
                        den4sb = rt.tile(
                            [128, 4], F32, tag="d4", bufs=3, name=f"d4_{h}_{j}"
                        )
                        nc.vector.reciprocal(den4sb, den8[:, 0:8:2])
                        # evacuate av unnormalized right away: frees the av
                        # PSUM bank with NO dependency on the 1/den broadcast
                        # (which has ~5us of DMA latency); the normalization
                        # multiply runs in place on O^T later.
                        nc.vector.tensor_copy(OT[h][:, ch], av)
                        # 1/den broadcast along partitions via DRAM round trip
                        dfd = dram.tile([1, 512], F32, tag="dfd", name=f"df{h}_{j}")
                        nc.sync.dma_start(
                            dfd.rearrange("a (c p) -> p a c", p=128), den4sb
                        )
                        inv_b = invp.tile(
                            [128, 512], F32, tag="inv", name=f"inv{h}_{j}"
                        )
                        nc.gpsimd.dma_start(
                            inv_b, dfd[0:1, :].to_broadcast([128, 512])
                        )
                        nc.vector.tensor_mul(OT[h][:, ch], OT[h][:, ch], inv_b)
                    return emit

                s_block(0)
                if nst > 1:
                    s_block(1)
                # previous head's tail (final den/AV block, 1/den round
                # trip, O^T multiply) rides here, hidden behind our S blocks
                if pend:
                    pend.pop(0)()
                for st in range(nst):
                    if st + 2 < nst:
                        s_block(st + 2)
                    if lasth:
                        inject(2)
                    elif j == 3 and h == 0:
                        # late-half injects only: early op tiles would stall
                        # on h3's chunk-2 O^T normalization still in flight
                        if st >= 8:
                            inject(1)
                    elif st % 2 == 1 or st >= nst - 4:
                        inject(1)
                    if st >= 1:
                        den_av(st - 1)
                        if lasth and st - 1 >= 12 and st - 1 < 15:
                            tail_block(st - 1 - 12)
                pend.append(make_tail())

                if not (j == 3 and h == 0):
                    # 5 at j<3 so the feeder (incl. the last rope pair)
                    # drains spaced among attention ops, not back-to-back
                    inject(4 if j == 3 else 5)

            # chunk end: drain the feeder, then the last head's tail (for
            # j=3 the tail's fused adds read y01 tiles written by the last
            # feeder units - emission must precede them on the DVE queue)
            inject(len(feed))
            pend.pop(0)()


def build_nc():
    nc = bacc.Bacc("TRN2", target_bir_lowering=False, debug=False, num_devices=8)
    xt = nc.dram_tensor("xt", [D, T], BF16, kind="ExternalInput").ap()
    wq = nc.dram_tensor("wq", [128, KC * 512], BF16, kind="ExternalInput").ap()
    wk = nc.dram_tensor("wk", [128, KC * HD], BF16, kind="ExternalInput").ap()
    wv = nc.dram_tensor("wv", [128, KC * HD], BF16, kind="ExternalInput").ap()
    wo = nc.dram_tensor("wo", [NQ * HD, D], BF16, kind="ExternalInput").ap()
    cost = nc.dram_tensor("cost", [HD, T], BF16, kind="ExternalInput").ap()
    sint = nc.dram_tensor("sint", [HD, T], BF16, kind="ExternalInput").ap()
    maskm = nc.dram_tensor("maskm", [384, 128], BF16, kind="ExternalInput").ap()
    y = nc.dram_tensor("y", [T, D], BF16, kind="ExternalOutput").ap()
    with tile.TileContext(nc) as tc:
        _body(tc, xt, wq, wk, wv, wo, cost, sint, maskm, y)
    nc.compile()
    return nc


def rope_tables():
    inv_freq = 1.0 / (10000.0 ** (np.arange(0, HD, 2, dtype=np.float32) / HD))
    t = np.arange(T, dtype=np.float32)
    freqs = t[:, None] * inv_freq[None, :]
    emb = np.concatenate([freqs, freqs], axis=1)  # [T, 128]
    cos = np.ascontiguousarray(np.cos(emb).T).astype(np.float32)
    sin = np.ascontiguousarray(np.sin(emb).T).astype(np.float32)
    sins = sin.copy()
    sins[0:64] = -sins[0:64]
    return cos, sins


def causal_mask_tile():
    # PE-mask pair + rotate-half matrix, stacked [384, 128]:
    # trineg[c, p] = -1e30 for c <= p, tri2[c, q] = 1 for c > q; their
    # matmul product trineg^T @ tri2 is -1e30 * (p - q) for p > q (t < s:
    # masked) and 0 otherwise. swapid is the rotate-half permutation
    # (lhsT: out[m] = in[(m + 64) % 128], sign folded into sint).
    cc = np.arange(128)
    trineg = np.where(cc[:, None] <= cc[None, :], -1.0e30, 0.0)
    tri2 = np.where(cc[:, None] > cc[None, :], 1.0, 0.0)
    sw = np.zeros((128, 128))
    sw[(np.arange(128) + 64) % 128, np.arange(128)] = 1.0
    return np.concatenate([trineg, tri2, sw], axis=0).astype(np.float32)


def _pack_w(w):
    """[KC*128, M] -> [128, KC*M] with packed[p, kc*M+m] = w[kc*128+p, m]."""
    kc, m = w.shape[0] // 128, w.shape[1]
    return np.ascontiguousarray(
        w.reshape(kc, 128, m).transpose(1, 0, 2).reshape(128, kc * m)
    )


def make_in_maps(x, Wq, Wk, Wv, Wo):
    scale = np.float32(1.0 / math.sqrt(HD))
    cos, sins = rope_tables()
    mask = causal_mask_tile()
    in_maps = []
    for c in range(8):
        b, g = c // 4, c % 4
        in_maps.append(
            {
                "xt": np.ascontiguousarray(x[b].T).astype(BFNP),
                "wq": _pack_w(Wq[:, 512 * g : 512 * (g + 1)] * (scale * 16)).astype(
                    BFNP
                ),
                "wk": _pack_w(Wk[:, 128 * g : 128 * (g + 1)] * 16).astype(BFNP),
                "wv": _pack_w(Wv[:, 128 * g : 128 * (g + 1)]).astype(BFNP),
                "wo": np.ascontiguousarray(Wo[512 * g : 512 * (g + 1), :]).astype(BFNP),
                "cost": cos.astype(BFNP),
                "sint": sins.astype(BFNP),
                "maskm": mask.astype(BFNP),
            }
        )
    return in_maps


_CACHE = {}


def _get_nc():
    if "nc" not in _CACHE:
        _CACHE["nc"] = build_nc()
    return _CACHE["nc"]


def kernel(**inputs):
    x = np.asarray(inputs["x"], np.float32)
    Wq = np.asarray(inputs["Wq"], np.float32)
    Wk = np.asarray(inputs["Wk"], np.float32)
    Wv = np.asarray(inputs["Wv"], np.float32)
    Wo = np.asarray(inputs["Wo"], np.float32)
    in_maps = make_in_maps(x, Wq, Wk, Wv, Wo)
    nc = _get_nc()
    res = run_bass_kernel_spmd(nc, in_maps, core_ids=list(range(8)))
    outs = [np.asarray(r["y"], dtype=np.float32) for r in res.results]
    y = np.stack(
        [
            outs[0] + outs[1] + outs[2] + outs[3],
            outs[4] + outs[5] + outs[6] + outs[7],
        ]
    )
    return y.astype(np.float32)


# revision 75
# speedup vs baseline: 1.0070x; 1.0070x over previous
"""GroupedQueryAttention Trainium2 kernel (bf16 + one-sided-fp8 S).

Sharding: 8 cores = 2 (batch) x 4 (kv-head groups / tensor parallel).
Core c: b = c//4, g = c%4 owns q-heads 4g..4g+3 and kv-head g.
Each core computes a partial o-projection (its 512 rows of Wo); the host
sums the 4 partials per batch (the "all-reduce" of the TP group).

Design notes (cost-model driven; ~188us of PE busy, the roofline for
this decomposition):
  - Everything bf16 on the wire and in SBUF (PSUM fp32): halves DMA
    traffic/SBUF pressure, and matmuls run 1 cyc/row at any ap size.
  - S matmuls in fp8 e4m3 DoubleRow (0.5 cyc/row): k is stored as an
    fp8 hi+lo pair occupying the two DoubleRow weight slots (so k
    carries NO quantization error), q as a single fp8 tile broadcast
    into both rhs slots via a 0-stride AP. q/k are pre-scaled by 16 on
    the host (fp8 range centering); the exp undoes the 256x with its
    scale operand. Measured end-to-end rel-err ~1.45e-2 (gate 2e-2);
    fp8 anywhere else (AV/proj/o-proj, even one-sided) breaks the gate.
  - v projected directly into natural [s, d] layout (lhsT = x block,
    rhs = Wv tile, ap=128): no PE transposes. All 4 column chains share
    ONE psum accumulation group (a bank holds one pending-zero region).
  - causal mask accumulated ON THE PE: trineg^T @ tri2 = -1e30*(s-t)
    for t < s, added into the diagonal S block - exp follows the S
    matmul immediately with no cross-engine mask hop.
  - RoPE rotate-half via a PE permutation matmul (swapid). DVE cannot
    cross partitions on real HW (CoreSim permits it; the BIR verifier
    does not). The rot matmul is emitted 2 feeder slots after its evac
    copy so the PE never waits on the copy engine.
  - each head's final den/AV block + reciprocal + O^T evac are deferred
    into the NEXT head's stream; av is evacuated unnormalized (frees
    the PSUM bank without waiting the ~5us 1/den DRAM-round-trip
    broadcast) and normalized in place later.
  - o-proj tail fully pipelined: (h0..h2) partials of the last 4
    t-tiles stage to SBUF mid-chunk-3; h3/chunk-3's AV + denominator
    close per 128-column block at s-blocks 12..15, so each tail row
    runs inside h3's attention as one matmul + one fused DVE
    scalar_tensor_tensor (yr = yp*(1/den) + y01).
  - DMA: ~4KB/partition pieces spread over SP/Act HWDGE and the Pool
    SWDGE path (which bypasses both the SP sequencer serialization and
    the shared HWDGE generator); y staged per t-row; final row flushed
    per-nch on two queues so the last transfer is small.
"""

import math
import sys

import ml_dtypes
import numpy as np

sys.path.insert(0, "/opt/trn_rl_repo")

import concourse.bass as bass  # noqa: E402
import concourse.tile as tile  # noqa: E402
from concourse import bacc, mybir  # noqa: E402
from concourse.bass_utils import run_bass_kernel_spmd  # noqa: E402

B, T, D = 2, 2048, 2048
NH, NKV, HD = 16, 4, 128
NQ = NH // NKV  # q heads per core
KC = D // 128  # contraction chunks
NT = T // 128  # t tiles
NJ = T // 512  # t chunks
F32 = mybir.dt.float32
BF16 = mybir.dt.bfloat16
F8 = mybir.dt.float8e4
DR = mybir.MatmulPerfMode.DoubleRow
X = mybir.AxisListType.X
EXP = mybir.ActivationFunctionType.Exp
COPY = mybir.ActivationFunctionType.Copy
BFNP = ml_dtypes.bfloat16
# q/k are pre-scaled by 16 on the host (fp8 range centering); S comes out
# scaled by 256, undone by the exp's scale operand.
SEXP = 1.0 / 256.0


def _body(tc, xt, wq, wk, wv, wo, cost_d, sint_d, maskm_d, y_d):
    nc = tc.nc
    from contextlib import ExitStack

    with ExitStack() as ctx:
        consts = ctx.enter_context(tc.tile_pool(name="consts", bufs=1))
        wpool = ctx.enter_context(tc.tile_pool(name="wpool", bufs=10))
        seq = ctx.enter_context(tc.tile_pool(name="seq", bufs=1))
        blk = ctx.enter_context(tc.tile_pool(name="blk", bufs=2))
        ptp = ctx.enter_context(tc.tile_pool(name="ptp", bufs=5))
        rt = ctx.enter_context(tc.tile_pool(name="rt", bufs=2))
        invp = ctx.enter_context(tc.tile_pool(name="invp", bufs=4))
        dram = ctx.enter_context(tc.tile_pool(name="dram", bufs=4, space="DRAM"))
        ps = ctx.enter_context(tc.tile_pool(name="ps", bufs=3, space="PSUM"))

        # causal mask as a PE accumulation: trineg^T @ tri2 adds
        # -1e30*(s-t) to the diagonal block where t < s, 0 elsewhere
        trineg = consts.tile([128, 128], BF16, tag="trineg")
        tri2 = consts.tile([128, 128], BF16, tag="tri2")
        swapid = consts.tile([128, 128], BF16, tag="swapid")
        onesr = consts.tile([128, 2], BF16, tag="onesr")
        nc.vector.memset(onesr, 1.0)

        wkt = wpool.tile([128, 16, 128], BF16, tag="w", name="wkt")
        wvt = wpool.tile([128, 16, 128], BF16, tag="w", name="wvt")
        wqt = [
            wpool.tile([128, 4, 512], BF16, tag="w", name=f"wq{i}") for i in range(4)
        ]
        wkr = wk.rearrange("p (c m) -> p c m", m=128)
        wvr = wv.rearrange("p (c m) -> p c m", m=128)
        cost = consts.tile([128, T], BF16, tag="cost")
        sint = consts.tile([128, T], BF16, tag="sint")

        # chunk-0 x: one [128, 16, 512] tile, loaded in staggered pieces so
        # the prologue chains (k lag 0, v lag 2, q lag 6) track the stream.
        # Small first pieces + SP/Act/Pool queue spreading minimize the time
        # to the first matmul (~3.9us: seq + HWDGE + DGE + transfer + sem).
        xw0 = blk.tile([128, 16, 512], BF16, tag="blk", name="xw0")
        xr0 = xt.rearrange("(c p) m -> p c m", p=128)[:, :, 0:512]
        wqr = [
            wq[:, 2048 * g : 2048 * (g + 1)].rearrange("p (c m) -> p c m", m=512)
            for g in range(4)
        ]
        # Pool's SWDGE path bypasses the shared HWDGE descriptor generator,
        # which serializes at ~625ns/DMA and is the real start constraint.
        # x pieces ride the Pool SWDGE path: it bypasses both the SP
        # sequencer serialization (~1.2us/DMA) and the shared HWDGE
        # descriptor generator (~625ns/DMA)
        nc.sync.dma_start(wkt[:, 0:2, :], wkr[:, 0:2, :])
        nc.scalar.dma_start(xw0[:, 0:2, :], xr0[:, 0:2, :])
        nc.gpsimd.dma_start(xw0[:, 2:4, :], xr0[:, 2:4, :])
        nc.sync.dma_start(wkt[:, 2:16, :], wkr[:, 2:16, :])
        nc.gpsimd.dma_start(xw0[:, 4:8, :], xr0[:, 4:8, :])
        nc.sync.dma_start(wqt[0], wqr[0])
        nc.gpsimd.dma_start(wvt[:, 0:4, :], wvr[:, 0:4, :])
        nc.gpsimd.dma_start(xw0[:, 8:12, :], xr0[:, 8:12, :])
        nc.sync.dma_start(wvt[:, 4:16, :], wvr[:, 4:16, :])
        nc.gpsimd.dma_start(xw0[:, 12:16, :], xr0[:, 12:16, :])
        nc.scalar.dma_start(wqt[1], wqr[1])
        nc.gpsimd.dma_start(wqt[2], wqr[2])
        nc.gpsimd.dma_start(wqt[3], wqr[3])
        nc.scalar.dma_start(trineg, maskm_d[0:128, :])
        nc.scalar.dma_start(tri2, maskm_d[128:256, :])
        nc.scalar.dma_start(swapid, maskm_d[256:384, :])
        nc.sync.dma_start(cost, cost_d)
        nc.sync.dma_start(sint, sint_d)

        # Activation-table preload: a dummy exp right after the DMA configs
        # so the one-time 1.28us table load runs during the initial stream-in
        # instead of in front of the first real exp.
        scratch = consts.tile([128, 1], F32, tag="scr")
        nc.vector.memset(scratch, 0.0)
        nc.scalar.activation(scratch, scratch, EXP)

        # q in single fp8 (DoubleRow rhs broadcasts it to both slots);
        # k as an fp8 hi+lo pair occupying the two DoubleRow weight slots,
        # so S matmuls run at 0.5 cycles/row with only q's quantization error
        qT8 = [seq.tile([128, T], F8, tag=f"qT{h}", name=f"qT{h}") for h in range(NQ)]
        kT8 = seq.tile([128, 2, T], F8, tag="kT", name="kT8")
        OT = [seq.tile([128, T], BF16, tag=f"ot{h}", name=f"ot{h}") for h in range(NQ)]
        vnat = seq.tile([128, T], BF16, tag="vnat", name="vnat")
        wot = []

        def wslc(m, kc):
            if m == "k":
                return wkt[:, kc, :]
            h = int(m[1])
            return wqt[kc // 4][:, kc % 4, 128 * h : 128 * (h + 1)]

        # ---------- emission units (proj chains, evacs, o-proj tiles) ------
        def chain_part(j, m, ref, xw, lo, hi):
            def emit():
                if lo == 0:
                    ref["pm"] = ps.tile(
                        [128, 512], F32, tag="pm", bufs=2, name=f"pm{j}_{m}"
                    )
                pm = ref["pm"]
                for kc in range(lo, hi):
                    if m == "v":
                        # one psum group for all 4 column chains (a bank can
                        # hold only one pending-zero region at a time)
                        for c in range(4):
                            nc.tensor.matmul(
                                pm[:, 128 * c : 128 * (c + 1)],
                                xw[:, kc, 128 * c : 128 * (c + 1)],
                                wvt[:, kc, :],
                                start=(kc == 0 and c == 0),
                                stop=(kc == KC - 1 and c == 3),
                            )
                    else:
                        nc.tensor.matmul(
                            pm,
                            wslc(m, kc),
                            xw[:, kc, :],
                            start=(kc == 0),
                            stop=(kc == KC - 1),
                        )
            return emit

        def rope_copy(j, m, ref, act=True):
            # always on Act: the rot matmul (PE) waits on this copy, and the
            # DVE queue is the congested one mid-chunk
            def emit():
                pm = ref["pm"]
                t0 = rt.tile([128, 512], BF16, tag="t0", bufs=6, name=f"t0_{j}_{m}")
                nc.scalar.activation(t0, pm, COPY)
                ref["t0"] = t0
            return emit

        def rope_rest(j, m, ref):
            # separate feeder unit: the rot matmul lands one injection slot
            # after the copy, so the PE never waits on the copy's engine
            def emit():
                ch = slice(512 * j, 512 * (j + 1))
                t0 = ref["t0"]
                # rotate-half on the PE (the DVE cannot cross partitions);
                # the sign of the rotation is folded into sint
                rot = ps.tile([128, 512], F32, tag="pm", bufs=2, name=f"ro{j}_{m}")
                nc.tensor.matmul(rot, swapid, t0)
                tc_ = rt.tile([128, 512], BF16, tag="t2", bufs=6, name=f"tc_{j}_{m}")
                nc.gpsimd.tensor_mul(tc_, t0, cost[:, ch])
                tmp = rt.tile([128, 512], BF16, tag="t1", bufs=6, name=f"t1_{j}_{m}")
                nc.vector.tensor_mul(tmp, rot, sint[:, ch])
                if m == "k":
                    ktb = rt.tile([128, 512], BF16, tag="ktb", bufs=2, name=f"kb{j}")
                    nc.vector.tensor_add(ktb, tc_, tmp)
                    # fp8 hi/lo split into the two DoubleRow weight slots
                    # (all DVE: keeps the Act queue clear for the exp stream)
                    nc.vector.tensor_copy(kT8[:, 0, ch], ktb)
                    ktr = rt.tile([128, 512], BF16, tag="ktr", bufs=2, name=f"kr{j}")
                    nc.vector.tensor_sub(ktr, ktb, kT8[:, 0, ch])
                    nc.vector.tensor_copy(kT8[:, 1, ch], ktr)
                else:
                    nc.vector.tensor_add(qT8[int(m[1])][:, ch], tc_, tmp)
            return emit

        def rope_evac(j, m, ref, act=False):
            def emit():
                rope_copy(j, m, ref, act)()
                rope_rest(j, m, ref)()
            return emit

        def v_evac(j, ref):
            def emit():
                ch = slice(512 * j, 512 * (j + 1))
                nc.vector.tensor_copy(vnat[:, ch], ref["pm"])
            return emit

        def wo_load(hh):
            def emit():
                w = wpool.tile([128, T], BF16, tag="w", name=f"wo{hh}")
                nc.sync.dma_start(w, wo[128 * hh : 128 * (hh + 1), :])
                wot.append(w)
            return emit

        # y staged per t-row; one 4KB/partition DMA per row on the Act queue
        yrow = {}

        def get_yrow(it):
            if it not in yrow:
                yrow[it] = rt.tile([128, T], BF16, tag="yrow", bufs=5, name=f"yr{it}")
            return yrow[it]

        def yflush(it):
            # SP is idle during chunk 3 (no x loads): row flushes go there
            nc.sync.dma_start(y_d[128 * it : 128 * (it + 1), :], yrow.pop(it))

        def oproj_tile(it, nch):
            def emit():
                yp = ps.tile([128, 512], F32, tag="pm", bufs=2, name=f"yp{it}_{nch}")
                for hh in range(4):
                    nc.tensor.matmul(
                        yp,
                        OT[hh][:, 128 * it : 128 * (it + 1)],
                        wot[hh][:, 512 * nch : 512 * (nch + 1)],
                        start=(hh == 0),
                        stop=(hh == 3),
                    )
                yr = get_yrow(it)
                # 3:1 Act/DVE split: late chunk-3 DVE is congested with the
                # pipelined tail's fused adds while Act's exps are tapering
                if nch != 3:
                    nc.scalar.activation(yr[:, 512 * nch : 512 * (nch + 1)], yp, COPY)
                else:
                    nc.vector.tensor_copy(yr[:, 512 * nch : 512 * (nch + 1)], yp)
                if nch == 3:
                    yflush(it)
            return emit

        # tail split: (h0,h1,h2) partials of the last 4 t-tiles staged to
        # SBUF mid-chunk-3; the h3 term is pipelined INTO h3's attention:
        # h3/chunk-3's AV and denominator complete per 128-column block at
        # s-blocks 12..15, so each tail row runs right after its block
        # closes, fused as yr = yp*(1/den) + y01 in one DVE op.
        y01 = {}

        def oproj01_tile(it, nch):
            def emit():
                yp = ps.tile([128, 512], F32, tag="pm", bufs=2, name=f"ya{it}_{nch}")
                for hh in range(3):
                    nc.tensor.matmul(
                        yp,
                        OT[hh][:, 128 * it : 128 * (it + 1)],
                        wot[hh][:, 512 * nch : 512 * (nch + 1)],
                        start=(hh == 0),
                        stop=(hh == 2),
                    )
                if nch % 2 == 0:
                    nc.scalar.activation(y01[(it, nch)], yp, COPY)
                else:
                    nc.vector.tensor_copy(y01[(it, nch)], yp)
            return emit
            return emit

        def proj_units(j, xw):
            # each chain's rope tail (rot matmul etc.) is deferred past the
            # NEXT chain's first part so its evac copy has ~2 injection
            # slots of lead before the PE needs it
            units = []
            pending = None
            for m in ["k", "v", "q0", "q1", "q2", "q3"]:
                ref = {}
                for i, lo in enumerate(range(0, KC, 4)):
                    units.append(chain_part(j, m, ref, xw, lo, lo + 4))
                    if i == 0 and pending is not None:
                        units.append(pending)
                        pending = None
                if m == "v":
                    units.append(v_evac(j, ref))
                else:
                    units.append(rope_copy(j, m, ref))
                    pending = rope_rest(j, m, ref)
                if j == 2:
                    if m == "k":
                        units.append(wo_load(0))
                    elif m == "v":
                        units.append(wo_load(1))
                    elif m == "q0":
                        units.append(wo_load(2))
                    elif m == "q1":
                        units.append(wo_load(3))
            units.append(pending)
            return units

        # ---------- prologue: proj(0) chains interleaved with the x/w
        # stream-in; q1/q2/q3 borrow the idle av/den PSUM tags
        MS = ["k", "v", "q0", "q1", "q2", "q3"]
        QS = ["q0", "q1", "q2", "q3"]
        ptag = {"k": ("pm", 2), "v": ("pm", 2), "q0": ("ps", 2),
                "q1": ("av", 2), "q2": ("av", 2), "q3": ("den", 2)}
        refs = {m: {} for m in MS}
        for m in MS:
            tg, nb = ptag[m]
            refs[m]["pm"] = ps.tile(
                [128, 512], F32, tag=tg, bufs=nb, name=f"pm0_{m}"
            )

        def mm0(m, kc):
            pm = refs[m]["pm"]
            if m == "v":
                for c in range(4):
                    nc.tensor.matmul(
                        pm[:, 128 * c : 128 * (c + 1)],
                        xw0[:, kc, 128 * c : 128 * (c + 1)],
                        wvt[:, kc, :],
                        start=(kc == 0 and c == 0),
                        stop=(kc == KC - 1 and c == 3),
                    )
            else:
                nc.tensor.matmul(
                    pm,
                    wslc(m, kc),
                    xw0[:, kc, :],
                    start=(kc == 0),
                    stop=(kc == KC - 1),
                )

        # k lags 0, v lags 5, q lags 9 behind the x/w arrival stream
        for kc in range(KC):
            mm0("k", kc)
            if kc >= 5:
                mm0("v", kc - 5)
            if kc >= 9:
                for m in QS:
                    mm0(m, kc - 9)
        # chain-major flush: each chain's rope evac is emitted the moment
        # its chain closes, overlapping the remaining flush matmuls, so
        # kT8/qT8 are ready well before attention starts. (k's rot must
        # follow v's evac: the pm ring would otherwise make the rot wait
        # on v's still-accumulating psum from inside the PE queue.)
        for kc in range(KC - 5, KC):
            mm0("v", kc)
        v_evac(0, refs["v"])()
        rope_evac(0, "k", refs["k"], act=True)()
        prev_rest = None
        for m in QS:
            for kc in range(KC - 9, KC):
                mm0(m, kc)
            if prev_rest is not None:
                prev_rest()
            rope_copy(0, m, refs[m])()
            prev_rest = rope_rest(0, m, refs[m])
        prev_rest()
        pro_evacs = []

        # ---------- attention per chunk, feeder interleaves next-chunk work
        pend = []
        for j in range(NJ):
            ch = slice(512 * j, 512 * (j + 1))
            nst = 4 * j + 4

            if j < 3:
                xw = blk.tile([128, 16, 512], BF16, tag="blk", name=f"xw{j + 1}")
                xr = xt.rearrange("(c p) m -> p c m", p=128)[
                    :, :, 512 * (j + 1) : 512 * (j + 2)
                ]
                for piece in range(4):
                    nc.sync.dma_start(
                        xw[:, 4 * piece : 4 * piece + 4, :],
                        xr[:, 4 * piece : 4 * piece + 4, :],
                    )
                feed = proj_units(j + 1, xw)
                if j == 0:
                    feed = pro_evacs + feed
            else:
                # (h0..h2) partials of the tail t-tiles sit last in the
                # feed: OT2's chunk-3 columns are ready early in h3, where
                # these inject (h3 runs inject(2) per s-block). y01 tiles
                # are pre-allocated so the pipelined tail can reference
                # them before the writes are emitted.
                for it in range(12, NT):
                    for nch in range(4):
                        y01[(it, nch)] = rt.tile(
                            [128, 512], BF16, tag="y01", bufs=16,
                            name=f"yA{it}_{nch}",
                        )
                op = [oproj_tile(it, nch) for it in range(12) for nch in range(4)]
                ya = [oproj01_tile(it, nch) for it in range(12, NT) for nch in range(4)]
                feed = op + ya

            def inject(n):
                for _ in range(n):
                    if feed:
                        feed.pop(0)()

            for h in range(NQ):
                den8 = ps.tile(
                    [128, 8],
                    F32,
                    tag="den",
                    bufs=2,
                    padded_shape=[128, 512],
                    name=f"den{h}_{j}",
                )
                av = ps.tile([128, 512], F32, tag="av", bufs=2, name=f"av{h}_{j}")
                pts = [None] * nst

                def s_block(st):
                    off = 128 * (st - 4 * j)
                    lo = max(0, off)
                    w = 512 - lo
                    sps = ps.tile(
                        [128, 512], F32, tag="ps", bufs=2, name=f"s{h}_{j}_{st}"
                    )
                    qb = (
                        qT8[h][:, 512 * j + lo : 512 * (j + 1)]
                        .rearrange("p (o m) -> p o m", o=1)
                        .broadcast_to([128, 2, w])
                    )
                    nc.tensor.matmul(
                        sps[:, lo:512],
                        kT8[:, :, 128 * st : 128 * (st + 1)],
                        qb,
                        perf_mode=DR,
                        start=True,
                        stop=(off < 0),
                    )
                    if off >= 0:
                        # causal mask accumulated on the PE: adds
                        # -1e30*(s-t) where t < s inside the diagonal block
                        nc.tensor.matmul(
                            sps[:, off : off + 128],
                            trineg,
                            tri2,
                            start=False,
                            stop=True,
                        )
                    pt = ptp.tile([128, 512], BF16, tag="pt", name=f"pt{h}_{j}_{st}")
                    nc.scalar.activation(pt[:, lo:512], sps[:, lo:512], EXP, scale=SEXP)
                    pts[st] = pt

                lasth = j == 3 and h == 3

                def den_av(st, den8=den8, av=av, pts=pts, nst=nst, lasth=lasth):
                    for c in range(max(0, st - 4 * j), 4):
                        # for the pipelined last head, each den column chain
                        # closes at its own final contribution (st = 12+c)
                        stp = (
                            (st == 4 * j + c)
                            if lasth
                            else (st == nst - 1 and c == 3)
                        )
                        nc.tensor.matmul(
                            den8[:, 2 * c : 2 * c + 2],
                            pts[st][:, 128 * c : 128 * (c + 1)],
                            onesr,
                            start=(st == 0 and c == 0),
                            stop=stp,
                        )
                    c0 = max(0, 128 * (st - 4 * j))
                    nc.tensor.matmul(
                        av[:, c0:512],
                        vnat[:, 128 * st : 128 * (st + 1)],
                        pts[st][:, c0:512],
                        start=(st == 0),
                        stop=(st == nst - 1),
                    )

                def tail_block(b, den8=den8, av=av, ch=ch):
                    # one 128-token row of the j3/h3 o-proj tail: available
                    # as soon as den chain b and av columns [128b..] close
                    it = 12 + b
                    invb = rt.tile([128, 1], F32, tag="d4", bufs=3, name=f"i3_{b}")
                    nc.vector.reciprocal(invb, den8[:, 2 * b : 2 * b + 1])
                    # Act copy: the DVE queue holds the previous block's
                    # fused adds, and the yb matmuls wait on this evac
                    nc.scalar.activation(
                        OT[3][:, 128 * it : 128 * (it + 1)],
                        av[:, 128 * b : 128 * (b + 1)],
                        COPY,
                    )
                    for nch in range(4):
                        yp = ps.tile(
                            [128, 512], F32, tag="pm", bufs=2, name=f"yb{it}_{nch}"
                        )
                        nc.tensor.matmul(
                            yp,
                            OT[3][:, 128 * it : 128 * (it + 1)],
                            wot[3][:, 512 * nch : 512 * (nch + 1)],
                        )
                        yr = get_yrow(it)
                        sl = slice(512 * nch, 512 * (nch + 1))
                        nc.vector.scalar_tensor_tensor(
                            yr[:, sl],
                            yp,
                            invb,
                            y01[(it, nch)],
                            mybir.AluOpType.mult,
                            mybir.AluOpType.add,
                        )
                        if it < 15:
                            if nch == 3:
                                nc.sync.dma_start(
                                    y_d[128 * it : 128 * (it + 1), :], yrow.pop(it)
                                )
                        else:
                            # final row: per-nch flushes split over SP/Act
                            if nch % 2 == 0:
                                nc.sync.dma_start(
                                    y_d[128 * it : 128 * (it + 1), sl], yr[:, sl]
                                )
                            else:
                                nc.scalar.dma_start(
                                    y_d[128 * it : 128 * (it + 1), sl], yr[:, sl]
                                )
                            if nch == 3:
                                yrow.pop(it)

                def make_tail(h=h, j=j, nst=nst, den8=den8, av=av,
                              den_av=den_av, ch=ch, tail_block=tail_block,
                              lasth=lasth):
                    def emit():
                        den_av(nst - 1)
                        if lasth:
                            # final 128-token block of the pipelined tail
                            tail_block(3)
                            return
                        den4sb = rt.tile(
                            [128, 4], F32, tag="d4", bufs=3, name=f"d4_{h}_{j}"
                        )
                        nc.vector.reciprocal(den4sb, den8[:, 0:8:2])
                        # evacuate av unnormalized right away: frees the av
                        # PSUM bank with NO dependency on the 1/den broadcast
                        # (which has ~5us of DMA latency); the normalization
                        # multiply runs in place on O^T later.
                        nc.vector.tensor_copy(OT[h][:, ch], av)
                        # 1/den broadcast along partitions via DRAM round trip
                        dfd = dram.tile([1, 512], F32, tag="dfd", name=f"df{h}_{j}")
                        nc.sync.dma_start(
                            dfd.rearrange("a (c p) -> p a c", p=128), den4sb
                        )
                        inv_b = invp.tile(
                            [128, 512], F32, tag="inv", name=f"inv{h}_{j}"
                        )
                        nc.gpsimd.dma_start(
                            inv_b, dfd[0:1, :].to_broadcast([128, 512])
                        )
                        nc.vector.tensor_mul(OT[h][:, ch], OT[h][:, ch], inv_b)
                    return emit

                s_block(0)
                if nst > 1:
                    s_block(1)
                # previous head's tail (final den/AV block, 1/den round
                # trip, O^T multiply) rides here, hidden behind our S blocks
                if pend:
                    pend.pop(0)()
                for st in range(nst):
                    if st + 2 < nst:
                        s_block(st + 2)
                    if lasth:
                        inject(2)
                    elif j == 3 and h == 0:
                        # late-half injects only: early op tiles would stall
                        # on h3's chunk-2 O^T normalization still in flight
                        if st >= 8:
                            inject(1)
                    elif st % 2 == 1 or st >= nst - 4:
                        inject(1)
                    if st >= 1:
                        den_av(st - 1)
                        if lasth and st - 1 >= 12 and st - 1 < 15:
                            tail_block(st - 1 - 12)
                pend.append(make_tail())

                if not (j == 3 and h == 0):
                    # 5 at j<3 so the feeder (incl. the last rope pair)
                    # drains spaced among attention ops, not back-to-back
                    inject(4 if j == 3 else 5)

            # chunk end: drain the feeder, then the last head's tail (for
            # j=3 the tail's fused adds read y01 tiles written by the last
            # feeder units - emission must precede them on the DVE queue)
            inject(len(feed))
            pend.pop(0)()


def build_nc():
    nc = bacc.Bacc("TRN2", target_bir_lowering=False, debug=False, num_devices=8)
    xt = nc.dram_tensor("xt", [D, T], BF16, kind="ExternalInput").ap()
    wq = nc.dram_tensor("wq", [128, KC * 512], BF16, kind="ExternalInput").ap()
    wk = nc.dram_tensor("wk", [128, KC * HD], BF16, kind="ExternalInput").ap()
    wv = nc.dram_tensor("wv", [128, KC * HD], BF16, kind="ExternalInput").ap()
    wo = nc.dram_tensor("wo", [NQ * HD, D], BF16, kind="ExternalInput").ap()
    cost = nc.dram_tensor("cost", [HD, T], BF16, kind="ExternalInput").ap()
    sint = nc.dram_tensor("sint", [HD, T], BF16, kind="ExternalInput").ap()
    maskm = nc.dram_tensor("maskm", [384, 128], BF16, kind="ExternalInput").ap()
    y = nc.dram_tensor("y", [T, D], BF16, kind="ExternalOutput").ap()
    with tile.TileContext(nc) as tc:
        _body(tc, xt, wq, wk, wv, wo, cost, sint, maskm, y)
    nc.compile()
    return nc


def rope_tables():
    inv_freq = 1.0 / (10000.0 ** (np.arange(0, HD, 2, dtype=np.float32) / HD))
    t = np.arange(T, dtype=np.float32)
    freqs = t[:, None] * inv_freq[None, :]
    emb = np.concatenate([freqs, freqs], axis=1)  # [T, 128]
    cos = np.ascontiguousarray(np.cos(emb).T).astype(np.float32)
    sin = np.ascontiguousarray(np.sin(emb).T).astype(np.float32)
    sins = sin.copy()
    sins[0:64] = -sins[0:64]
    return cos, sins


def causal_mask_tile():
    # PE-mask pair + rotate-half matrix, stacked [384, 128]:
    # trineg[c, p] = -1e30 for c <= p, tri2[c, q] = 1 for c > q; their
    # matmul product trineg^T @ tri2 is -1e30 * (p - q) for p > q (t < s:
    # masked) and 0 otherwise. swapid is the rotate-half permutation
    # (lhsT: out[m] = in[(m + 64) % 128], sign folded into sint).
    cc = np.arange(128)
    trineg = np.where(cc[:, None] <= cc[None, :], -1.0e30, 0.0)
    tri2 = np.where(cc[:, None] > cc[None, :], 1.0, 0.0)
    sw = np.zeros((128, 128))
    sw[(np.arange(128) + 64) % 128, np.arange(128)] = 1.0
    return np.concatenate([trineg, tri2, sw], axis=0).astype(np.float32)


def _pack_w(w):
    """[KC*128, M] -> [128, KC*M] with packed[p, kc*M+m] = w[kc*128+p, m]."""
    kc, m = w.shape[0] // 128, w.shape[1]
    return np.ascontiguousarray(
        w.reshape(kc, 128, m).transpose(1, 0, 2).reshape(128, kc * m)
    )


def make_in_maps(x, Wq, Wk, Wv, Wo):
    scale = np.float32(1.0 / math.sqrt(HD))
    cos, sins = rope_tables()
    mask = causal_mask_tile()
    in_maps = []
    for c in range(8):
        b, g = c // 4, c % 4
        in_maps.append(
            {
                "xt": np.ascontiguousarray(x[b].T).astype(BFNP),
                "wq": _pack_w(Wq[:, 512 * g : 512 * (g + 1)] * (scale * 16)).astype(
                    BFNP
                ),
                "wk": _pack_w(Wk[:, 128 * g : 128 * (g + 1)] * 16).astype(BFNP),
                "wv": _pack_w(Wv[:, 128 * g : 128 * (g + 1)]).astype(BFNP),
                "wo": np.ascontiguousarray(Wo[512 * g : 512 * (g + 1), :]).astype(BFNP),
                "cost": cos.astype(BFNP),
                "sint": sins.astype(BFNP),
                "maskm": mask.astype(BFNP),
            }
        )
    return in_maps


_CACHE = {}


def _get_nc():
    if "nc" not in _CACHE:
        _CACHE["nc"] = build_nc()
    return _CACHE["nc"]


def kernel(**inputs):
    x = np.asarray(inputs["x"], np.float32)
    Wq = np.asarray(inputs["Wq"], np.float32)
    Wk = np.asarray(inputs["Wk"], np.float32)
    Wv = np.asarray(inputs["Wv"], np.float32)
    Wo = np.asarray(inputs["Wo"], np.float32)
    in_maps = make_in_maps(x, Wq, Wk, Wv, Wo)
    nc = _get_nc()
    res = run_bass_kernel_spmd(nc, in_maps, core_ids=list(range(8)))
    outs = [np.asarray(r["y"], dtype=np.float32) for r in res.results]
    y = np.stack(
        [
            outs[0] + outs[1] + outs[2] + outs[3],
            outs[4] + outs[5] + outs[6] + outs[7],
        ]
    )
    return y.astype(np.float32)
